# revision 31
# baseline (speedup 1.0000x reference)
"""AttentionBlock3D (GroupNorm + single-head self-attention + residual) on 8 TRN2 cores.

Sharding: core = (batch b in {0,1}) x (1024-row slice of the 4096 attention rows).
Each core redundantly computes its batch's GroupNorm stats and full K/V
(cheap), and attention + output projection for its own 1024 query rows.
No collectives. The host ROTATES each core's x copy so that its query rows
are always columns 0..1024 (attention is permutation-invariant over keys).

Math folding (all computed on-device from the real inputs; nothing assumes
zero biases):
  hn = x*A + B per channel, with A = gamma*rsqrt(var_g+eps), B = beta - mean_g*A
  q  = (Wq . A) x_q + cq           cq = Wq B + bq   (bias folded into q copy)
  k  = (Wk . A) x                  (k bias cancels in softmax over keys)
  v0 = (Wv . A) x                  cv = Wv B + bv   (rows of softmax sum to 1
                                   => P @ (cv 1^T) = cv 1^T, folded into bpe)
  S^T[m,n] = sum_o k[o,m] q[o,n];  E = exp(S/16);  r[n] = sum_m E[m,n]
  out = (x_q + bpe) + ((Wp^T)^T @ (E^T V)) * (1/r),  bpe = bp + Wp cv
"""

import os
import numpy as np
from contextlib import ExitStack

C = 256          # channels
N = 4096         # spatial positions (16*16*16)
NQ = 1024        # query rows per core
GROUPS = 8
GSIZE = C // GROUPS
EPS = 1e-5
NCH = NQ // 512  # n-chunks per core

_CACHE = {}
LAST_RESULTS = None  # test harness can inspect trace results


def _build_nc(use_f32r=True):
    import concourse.bacc as bacc
    import concourse.bass as bass
    import concourse.tile as tile
    from concourse import mybir

    f32 = mybir.dt.float32
    f32r = mybir.dt.float32r
    AF = mybir.ActivationFunctionType

    fr = f32r if use_f32r else f32
    xbf16 = os.environ.get("BASSK_XBF16", "1") == "1"
    bx = mybir.dt.bfloat16 if xbf16 else fr

    def R(ap):
        return ap

    nc = bacc.Bacc("TRN2", target_bir_lowering=False, debug=False,
                   enable_asserts=False)

    # ---- DRAM I/O (per-core) ----
    xb_d = nc.dram_tensor("xb", [C, N],
                          mybir.dt.bfloat16 if os.environ.get("BASSK_XBF16", "1") == "1" else f32,
                          kind="ExternalInput").ap()
    xq_d = nc.dram_tensor("xq", [C, NQ], f32, kind="ExternalInput").ap()
    wall_d = nc.dram_tensor("wall", [C, 4 * C], f32, kind="ExternalInput").ap()
    small_d = nc.dram_tensor("small", [C, 5 + GROUPS], f32, kind="ExternalInput").ap()
    gmask8_d = nc.dram_tensor("gmask8", [GROUPS, C], f32, kind="ExternalInput").ap()
    out_d = nc.dram_tensor("out", [C, NQ], f32, kind="ExternalOutput").ap()

    with tile.TileContext(nc) as tc, ExitStack() as ctx:
        big = ctx.enter_context(tc.tile_pool(name="big", bufs=1))
        consts = ctx.enter_context(tc.tile_pool(name="consts", bufs=1))
        work = ctx.enter_context(tc.tile_pool(name="work", bufs=3))
        pw = ctx.enter_context(tc.tile_pool(name="pw", bufs=3, space="PSUM"))
        pacc = ctx.enter_context(tc.tile_pool(name="pacc", bufs=3, space="PSUM"))
        pr = ctx.enter_context(tc.tile_pool(name="pr", bufs=1, space="PSUM"))
        pstat = ctx.enter_context(tc.tile_pool(name="pstat", bufs=1, space="PSUM"))

        # ---- constants / small loads (before the big x load) ----
        ones_f32 = consts.tile([128, 128], f32)
        nc.vector.memset(ones_f32, 1.0)
        ones128 = consts.tile([128, 128], fr)
        nc.vector.tensor_copy(ones128, ones_f32)
        # eps8 = Sqrt(EPS^2) on ACT: forces the Sqrt act-table load to run at
        # t~0 (gsd depends on eps8, so the scheduler cannot sink it)
        eps_sq = consts.tile([GROUPS, 1], f32)
        nc.vector.memset(eps_sq, EPS * EPS)
        eps8 = consts.tile([GROUPS, 1], f32)
        nc.scalar.activation(out=eps8, in_=eps_sq, func=AF.Sqrt, scale=1.0)

        # ---- load x first (chunked, stats interleaved), then consts/weights ----
        xb_sb = []
        stats_l = []
        for ct in range(2):
            cs = slice(ct * 128, (ct + 1) * 128)
            t = big.tile([128, N], bx, name=f"xb_sb{ct}")
            stats = work.tile([128, 8, 6], f32, name="stats", tag="stats")
            for s in range(2):
                fs = slice(s * 2048, (s + 1) * 2048)
                nc.sync.dma_start(out=t[:, fs],
                                  in_=xb_d[cs, fs] if xbf16 else xb_d[cs, fs].bitcast(fr))
                for s2 in range(4):
                    ss = slice(s * 2048 + s2 * 512, s * 2048 + (s2 + 1) * 512)
                    nc.vector.bn_stats(out=stats[:, s * 4 + s2, :],
                                       in_=t[:, ss] if xbf16 else t[:, ss].bitcast(f32))
            xb_sb.append(t)
            stats_l.append(stats)
        xq = []
        for ct in range(2):
            cs = slice(ct * 128, (ct + 1) * 128)
            t = big.tile([128, NQ], f32, name=f"xq_sb{ct}")
            nc.sync.dma_start(out=t, in_=xq_d[cs, :])
            xq.append(t)

        small_sb, wall_sb = [], []
        for ct in range(2):
            cs = slice(ct * 128, (ct + 1) * 128)
            t = consts.tile([128, 5 + GROUPS], f32, name=f"small_sb{ct}")
            nc.sync.dma_start(out=t, in_=small_d[cs, :]); small_sb.append(t)
        gmask8_sb = consts.tile([GROUPS, C], f32)
        nc.sync.dma_start(out=gmask8_sb, in_=gmask8_d)
        for ct in range(2):
            cs = slice(ct * 128, (ct + 1) * 128)
            t = consts.tile([128, 4 * C], f32, name=f"wall_sb{ct}")
            nc.sync.dma_start(out=t, in_=wall_d[cs, :]); wall_sb.append(t)
        gamma_sb = [t[:, 0:1] for t in small_sb]
        beta_sb = [t[:, 1:2] for t in small_sb]
        bq_sb = [t[:, 2:3] for t in small_sb]
        bv_sb = [t[:, 3:4] for t in small_sb]
        bp_sb = [t[:, 4:5] for t in small_sb]
        gmask_sb = [t[:, 5:5 + GROUPS] for t in small_sb]
        wqt_sb = [t[:, 0 * C:1 * C] for t in wall_sb]
        wkt_sb = [t[:, 1 * C:2 * C] for t in wall_sb]
        wvt_sb = [t[:, 2 * C:3 * C] for t in wall_sb]
        wpt_sb = [t[:, 3 * C:4 * C] for t in wall_sb]

        # per-channel moments -> group sums via 0/1 mask matmul (exact fp32)
        gp = pstat.tile([GROUPS, 2], f32, tag="pstat")
        for ct in range(2):
            stile = work.tile([128, 2], f32, name="stile", tag="stile")
            msq = work.tile([128, 1], f32, name="msq", tag="msq")
            nc.vector.bn_aggr(out=stile, in_=stats_l[ct])
            nc.vector.tensor_mul(msq, stile[:, 0:1], stile[:, 0:1])
            nc.vector.tensor_add(stile[:, 1:2], stile[:, 1:2], msq)
            nc.tensor.matmul(gp, lhsT=gmask_sb[ct], rhs=stile,
                             start=(ct == 0), stop=(ct == 1))

        # ---- group stats -> per-channel A, B (PE mask8 broadcast, no DMA) ----
        gms = work.tile([GROUPS, 2], f32, name="gms")
        gvar = work.tile([GROUPS, 1], f32, name="gvar")
        gsd = work.tile([GROUPS, 1], f32, name="gsd")
        gsb = work.tile([GROUPS, 2], f32, name="gsb")
        nc.vector.tensor_scalar_mul(gms, gp, 1.0 / GSIZE)
        nc.vector.tensor_mul(gvar, gms[:, 0:1], gms[:, 0:1])
        nc.vector.tensor_sub(gvar, gms[:, 1:2], gvar)
        nc.scalar.activation(out=gsd, in_=gvar, func=AF.Sqrt, bias=eps8, scale=1.0)
        nc.vector.tensor_copy(gsb[:, 0:1], gms[:, 0:1])
        nc.vector.reciprocal(out=gsb[:, 1:2], in_=gsd)

        A_sb, B_sb = [], []
        for ct in range(2):
            gbp = pstat.tile([128, 2], f32, name="gbp", tag="pstat")
            nc.tensor.matmul(gbp, lhsT=gmask8_sb[:, ct * 128:(ct + 1) * 128],
                             rhs=gsb, start=True, stop=True)
            At = consts.tile([128, 1], f32, name=f"A_sb{ct}")
            Bt = consts.tile([128, 1], f32, name=f"B_sb{ct}")
            nc.vector.tensor_mul(At, gamma_sb[ct], gbp[:, 1:2])
            nc.vector.tensor_mul(Bt, gbp[:, 0:1], At)
            nc.vector.tensor_sub(Bt, beta_sb[ct], Bt)
            A_sb.append(At); B_sb.append(Bt)

        # ---- fold A into weights (split across DVE and ACT) ----
        wqte, wkte, wvte = [], [], []
        for lst, wsrc, nm in ((wkte, wkt_sb, "wkte"), (wqte, wqt_sb, "wqte"),
                              (wvte, wvt_sb, "wvte")):
            for ct in range(2):
                t = consts.tile([128, C], bx, name=f"{nm}{ct}")
                if ct == 0:
                    nc.vector.tensor_scalar_mul(t, wsrc[ct], A_sb[ct])
                else:
                    nc.scalar.activation(out=t, in_=wsrc[ct], func=AF.Identity,
                                         scale=A_sb[ct])
                lst.append(t)
        # preload the Exp act table while PE/DVE are busy with k/q/v
        dummy3 = consts.tile([1, 1], f32)
        nc.scalar.activation(out=dummy3, in_=A_sb[1][0:1, :], func=AF.Exp, scale=1.0)

        # ---- k = (Wk.A) x   [o, m] layout (first PE bulk work) ----
        k_sb = [big.tile([128, N], fr, name=f"k_sb{ot}") for ot in range(2)]
        q_sb = [big.tile([128, NQ], fr, name=f"q_sb{ot}") for ot in range(2)]
        for ot in range(2):
            os_ = slice(ot * 128, (ot + 1) * 128)
            for mc in range(8):
                fs = slice(mc * 512, (mc + 1) * 512)
                kp = pw.tile([128, 512], f32, name="kp", tag="pw")
                for ct in range(2):
                    nc.tensor.matmul(kp, lhsT=R(wkte[ct][:, os_]),
                                     rhs=R(xb_sb[ct][:, fs]),
                                     start=(ct == 0), stop=(ct == 1))
                if mc % 2 == 0:
                    nc.vector.tensor_copy(k_sb[ot][:, fs], kp)
                else:
                    nc.scalar.copy(k_sb[ot][:, fs], kp)

        # ---- bias vectors (PE cost tiny; overlaps with k copies) ----
        def bias_vec(wt_sb, rhs_tiles, badd, nm):
            outs = []
            for oh in range(2):
                p = pstat.tile([128, 1], f32, name=f"{nm}p", tag="pstat")
                for ct in range(2):
                    nc.tensor.matmul(p, lhsT=wt_sb[ct][:, oh * 128:(oh + 1) * 128],
                                     rhs=rhs_tiles[ct], start=(ct == 0), stop=(ct == 1))
                t = consts.tile([128, 1], f32, name=f"{nm}{oh}")
                nc.scalar.activation(out=t, in_=p, func=AF.Identity,
                                     bias=badd[oh], scale=1.0)
                outs.append(t)
            return outs

        cq_sb = bias_vec(wqt_sb, B_sb, bq_sb, "cq")
        cv_sb = bias_vec(wvt_sb, B_sb, bv_sb, "cv")
        bpe_sb = bias_vec(wpt_sb, cv_sb, bp_sb, "bpe")

        # f32r copy of Wp^T for the projection matmuls
        wpte = []
        for ct in range(2):
            t = consts.tile([128, C], fr, name=f"wpte{ct}")
            nc.vector.tensor_copy(t, wpt_sb[ct])
            wpte.append(t)

        # residual+bias base: xqb = x_q + bpe (off critical path)
        xqb = [big.tile([128, NQ], f32, name=f"xqb{ot}") for ot in range(2)]
        for ot in range(2):
            nc.gpsimd.tensor_scalar_add(xqb[ot], xq[ot], bpe_sb[ot])

        # ---- q = (Wq.A) x_q + cq ----
        for ot in range(2):
            os_ = slice(ot * 128, (ot + 1) * 128)
            for qc in range(NCH):
                fs = slice(qc * 512, (qc + 1) * 512)
                qp = pw.tile([128, 512], f32, name="qp", tag="pw")
                for ct in range(2):
                    nc.tensor.matmul(qp, lhsT=R(wqte[ct][:, os_]),
                                     rhs=R(xb_sb[ct][:, fs]),
                                     start=(ct == 0), stop=(ct == 1))
                if ot == 0:
                    nc.scalar.activation(out=q_sb[ot][:, fs], in_=qp,
                                         func=AF.Identity, bias=cq_sb[ot], scale=1.0)
                else:
                    nc.vector.tensor_scalar_add(q_sb[ot][:, fs], qp, cq_sb[ot])

        # ---- v = (Wv.A) x   [m, o] layout ----
        v_sb = big.tile([128, 32, C], fr, name="v_sb")
        for mt in range(32):
            ms = slice(mt * 128, (mt + 1) * 128)
            vp = pw.tile([128, C], f32, name="vp", tag="pw")
            for ct in range(2):
                nc.tensor.matmul(vp, lhsT=R(xb_sb[ct][:, ms]), rhs=R(wvte[ct]),
                                 start=(ct == 0), stop=(ct == 1))
            if mt % 2 == 0:
                nc.scalar.copy(v_sb[:, mt, :], vp)
            else:
                nc.vector.tensor_copy(v_sb[:, mt, :], vp)

        # ---- attention + projection, per n-chunk ----
        att_sb = [big.tile([128, NQ], fr, name=f"att_sb{ot}") for ot in range(2)]
        for nch in range(NCH):
            ns = slice(nch * 512, (nch + 1) * 512)
            otp = [pacc.tile([128, 512], f32, name=f"otp{oh}", tag="acc")
                   for oh in range(2)]
            rp = pr.tile([128, 512], f32, name="rp", tag="pr")
            for mt in range(32):
                ms = slice(mt * 128, (mt + 1) * 128)
                sp = pw.tile([128, 512], f32, name="sp", tag="pw")
                for ot in range(2):
                    nc.tensor.matmul(sp, lhsT=R(k_sb[ot][:, ms]),
                                     rhs=R(q_sb[ot][:, ns]),
                                     start=(ot == 0), stop=(ot == 1))
                e = work.tile([128, 512], fr, name="e", tag="e")
                nc.scalar.activation(out=e, in_=sp, func=AF.Exp, scale=1.0 / 16.0)
                nc.tensor.matmul(rp, lhsT=R(ones128), rhs=R(e),
                                 start=(mt == 0), stop=(mt == 31))
                for oh in range(2):
                    nc.tensor.matmul(otp[oh],
                                     lhsT=R(v_sb[:, mt, oh * 128:(oh + 1) * 128]),
                                     rhs=R(e), start=(mt == 0), stop=(mt == 31))
            rb = work.tile([128, 512], f32, name="rb", tag="rb", bufs=2)
            nc.vector.reciprocal(out=rb, in_=rp)
            # att = (E^T V) * (1/r): normalization folded into the psum drain
            for oh in range(2):
                nc.vector.tensor_mul(att_sb[oh][:, ns], otp[oh], rb)
            # projection + add-only epilogue for this chunk
            last = (nch == NCH - 1)
            st_engines = [nc.sync, nc.scalar] if last else [nc.sync, nc.sync]
            for ot in range(2):
                os_ = slice(ot * 128, (ot + 1) * 128)
                pp = pacc.tile([128, 512], f32, name="pp", tag="acc")
                for ct in range(2):
                    nc.tensor.matmul(pp, lhsT=R(wpte[ct][:, os_]),
                                     rhs=R(att_sb[ct][:, ns]),
                                     start=(ct == 0), stop=(ct == 1))
                ot_t = work.tile([128, 512], f32, name="ot_t", tag="ot_t")
                for hh in range(2):
                    hs = slice(hh * 256, (hh + 1) * 256)
                    ds = slice(nch * 512 + hh * 256, nch * 512 + (hh + 1) * 256)
                    nc.vector.tensor_add(ot_t[:, hs], pp[:, hs], xqb[ot][:, ds])
                st_engines[ot].dma_start(out=out_d[os_, ns], in_=ot_t)

    nc.compile()
    return nc


def _get_nc():
    key = "nc"
    if key not in _CACHE:
        _CACHE[key] = _build_nc(use_f32r=(os.environ.get("BASSK_F32R", "1") == "1"))
    return _CACHE[key]


def _host_inputs(x, gamma, beta, Wq, bq, Wk, bk, Wv, bv, Wp, bp):
    x = np.asarray(x, np.float32)
    xf = np.ascontiguousarray(x.reshape(2, C, N))
    gamma = np.asarray(gamma, np.float32).reshape(C, 1)
    beta = np.asarray(beta, np.float32).reshape(C, 1)
    wqt = np.ascontiguousarray(np.asarray(Wq, np.float32).T)
    wkt = np.ascontiguousarray(np.asarray(Wk, np.float32).T)
    wvt = np.ascontiguousarray(np.asarray(Wv, np.float32).T)
    wpt = np.ascontiguousarray(np.asarray(Wp, np.float32).T)
    bq = np.asarray(bq, np.float32).reshape(C, 1)
    bv = np.asarray(bv, np.float32).reshape(C, 1)
    bp = np.asarray(bp, np.float32).reshape(C, 1)
    gmask = np.zeros((C, GROUPS), np.float32)
    gmask[np.arange(C), np.arange(C) // GSIZE] = 1.0
    gmask8 = np.ascontiguousarray(gmask.T)
    wall = np.ascontiguousarray(np.hstack([wqt, wkt, wvt, wpt]))
    small = np.ascontiguousarray(np.hstack([gamma, beta, bq, bv, bp, gmask]))

    xbf16 = os.environ.get("BASSK_XBF16", "1") == "1"
    if xbf16:
        import ml_dtypes
    in_maps = []
    for core in range(8):
        b, j = divmod(core, 4)
        xrot = np.ascontiguousarray(np.roll(xf[b], -j * NQ, axis=1))
        in_maps.append({
            "xb": xrot.astype(ml_dtypes.bfloat16) if xbf16 else xrot,
            "xq": np.ascontiguousarray(xrot[:, :NQ]),
            "wall": wall, "small": small, "gmask8": gmask8,
        })
    return in_maps


def kernel(x, gamma, beta, Wq, bq, Wk, bk, Wv, bv, Wp, bp):
    from concourse.bass_utils import run_bass_kernel_spmd
    global LAST_RESULTS

    orig_shape = np.asarray(x).shape
    in_maps = _host_inputs(x, gamma, beta, Wq, bq, Wk, bk, Wv, bv, Wp, bp)
    nc = _get_nc()

    trace = os.environ.get("BASSK_TRACE", "0") == "1"
    res = run_bass_kernel_spmd(nc, in_maps, core_ids=list(range(8)), trace=trace)
    LAST_RESULTS = res

    out = np.empty((2, C, N), np.float32)
    for core in range(8):
        b, j = divmod(core, 4)
        out[b][:, j * NQ:(j + 1) * NQ] = res.results[core]["out"]
    return out.reshape(orig_shape)
